# revision 1
# baseline (speedup 1.0000x reference)
"""CARAFE-Downsample Trainium2 kernel v3 (8 NeuronCores, batch-parallel).

v2 -> v3: same math (kernel2.py / sim2.py), restructured to amortize
per-op overheads:
 - products batched over half-groups of 4 blocks: ONE DVE/GPSIMD
   tensor_tensor over [128, 4, 256] with pair-replicated bf16 weights
   (enables the DVE 2x packed mode despite broadcast weights).
 - PE accumulates with constant shift matrices in 2-block pair matmuls
   (N=512, one PSUM bank per pair); identity-group products and the
   direct-accumulator join share the identity stationary.
 - "direct" dj=0 taps bypass PE: batched product + batched TT add into
   a half-group accumulator, joined per pair by one identity matmul.
 - ACT builds a few per-block products and does all PSUM evacuations.
 - weights: per block 3 PE transposes (base/+1/-1 flat windows of the
   exp'd mask) -> PSUM bf16; DVE 3-variant reduce+recip; fold writes
   pair-replicated normalized weights into the group weight tile.
"""

import numpy as np
import ml_dtypes

import concourse.bass as bass
import concourse.bacc as bacc
import concourse.tile as tile
from concourse import mybir
from concourse.bass_utils import run_bass_kernel_spmd

B, C, H, W = 8, 256, 128, 128
CC, KK, HP, WP, NB = 64, 5, 64, 64, 32
NCORES = 8
NG = 4          # groups
DR = False      # fp8 DoubleRow mask conv (3D-free rhs wedges HW)
GB = 8          # blocks per group

BF16 = mybir.dt.bfloat16
F32 = mybir.dt.float32
FP8 = mybir.dt.float8e4
NPBF = ml_dtypes.bfloat16
NPF8 = ml_dtypes.float8_e4m3

_CPDJ = [(0, -1), (1, -1), (0, 0), (1, 0), (0, 1)]
_OHDH = [(0, -1), (1, -1), (0, 0), (1, 0), (0, 1)]
_VAR = {-1: 1, 0: 0, 1: 2}

# ---- tap assignment: v 18, a 4, g 3 (v3.2-best) ----
TAP_MODE = {}
_dj0 = [(i, j) for j in (2, 3) for i in range(5)]    # 10
_djm = [(i, j) for j in (0, 1) for i in range(5)]    # 10
_djp = [(i, 4) for i in range(5)]                    # 5
for t in _dj0:
    TAP_MODE[t] = "v"
for t in _djm[:5]:
    TAP_MODE[t] = "v"
for t in _djm[5:9]:
    TAP_MODE[t] = "a"
TAP_MODE[_djm[9]] = "g"
for t in _djp[:3]:
    TAP_MODE[t] = "v"
for t in _djp[3:]:
    TAP_MODE[t] = "g"
PAIRS = []


def _slot(kk, oh, cp):
    return ((kk + 1) * 2 + oh) * 2 + cp


def _build_nc():
    nc = bacc.Bacc(None, target_bir_lowering=False, debug=False)

    xall_d = nc.declare_dram_parameter("xall", [128, 136, C], BF16, isOutput=False)
    XCPL = 16912 if DR else 16900   # fp8 plane stride (16-aligned for DR)
    xcp_d = nc.declare_dram_parameter("xcp", [128, 2 * XCPL], FP8, isOutput=False)
    if DR:
        w2_d = nc.declare_dram_parameter("w2", [128, 9, 2, 32], FP8, isOutput=False)
    else:
        w2_d = nc.declare_dram_parameter("w2", [128, 18, 25], FP8, isOutput=False)
    b2_d = nc.declare_dram_parameter("b2", [25, 1], F32, isOutput=False)
    sc_d = nc.declare_dram_parameter("sc", [25, 1], F32, isOutput=False)
    id_d = nc.declare_dram_parameter("idn", [25, 25], BF16, isOutput=False)
    shm_d = nc.declare_dram_parameter("shm", [128, 3, 128], BF16, isOutput=False)
    out_d = nc.declare_dram_parameter("out", [128, NB * C], BF16, isOutput=True)

    taps = [(i, j) for i in range(5) for j in range(5)]
    prod_taps = [t for t in taps if TAP_MODE[t] != "direct"]
    mm_taps = ([t for t in prod_taps if _CPDJ[t[1]][1] == -1]
               + [t for t in prod_taps if _CPDJ[t[1]][1] == 1]
               + [t for t in prod_taps if _CPDJ[t[1]][1] == 0])

    with tile.TileContext(nc) as tc:
        with (
            tc.tile_pool(name="consts", bufs=1) as consts,
            tc.tile_pool(name="xbig", bufs=1) as xbig,
            tc.tile_pool(name="psM", bufs=2, space="PSUM") as psM,
            tc.tile_pool(name="psW", bufs=2, space="PSUM") as psW,
            tc.tile_pool(name="psP", bufs=4, space="PSUM") as psP,
            tc.tile_pool(name="wrep", bufs=4) as wrep,
            tc.tile_pool(name="wsb", bufs=4) as wsb,
            tc.tile_pool(name="prodp", bufs=14) as prodp,
            tc.tile_pool(name="accp", bufs=2) as accp,
        ):
            # ---- input DMAs ----
            xcp = xbig.tile([128, 2 * XCPL], FP8)
            nc.sync.dma_start(out=xcp, in_=xcp_d[:, :])
            if DR:
                w2_sb = consts.tile([128, 9, 2, 32], FP8)
                nc.sync.dma_start(out=w2_sb, in_=w2_d[:, :, :, :])
            else:
                w2_sb = consts.tile([128, 18, 25], FP8)
                nc.sync.dma_start(out=w2_sb, in_=w2_d[:, :, :])
            b2_sb = consts.tile([25, 1], F32)
            nc.sync.dma_start(out=b2_sb, in_=b2_d[:, :])
            sc_sb = consts.tile([25, 1], F32)
            nc.sync.dma_start(out=sc_sb, in_=sc_d[:, :])
            id_sb = consts.tile([25, 25], BF16)
            nc.sync.dma_start(out=id_sb, in_=id_d[:, :])
            shm_sb = consts.tile([128, 3, 128], BF16)
            nc.sync.dma_start(out=shm_sb, in_=shm_d[:, :, :])
            xall = xbig.tile([128, 136, C], BF16)
            nc.sync.dma_start(out=xall, in_=xall_d[:, :, :])

            outst0 = xbig.tile([128, 16 * C], BF16)
            outst1 = xbig.tile([128, 16 * C], BF16)

            m2g = []
            for g in range(NG):
                m2 = xbig.tile([25, 1026], BF16, name=f"m2_{g}")
                nc.vector.memset(m2[:, 0:1], 1.0)
                nc.vector.memset(m2[:, 1025:1026], 1.0)
                m2g.append(m2)

            # ---- helpers ----
            def emit_chunk(j2):
                pm = psM.tile([25, 512], F32, name=f"pm{j2}", tag="pm")
                ti = 0
                for di in range(3):
                    for dj in range(3):
                        for e in range(2):
                            off = e * XCPL + (16 * j2 + di) * 130 + dj
                            rhs = bass.AP(
                                tensor=xcp.tensor,
                                offset=xcp.offset + off,
                                ap=[xcp.ap[0], [260, 8], [2, 64]],
                            )
                            nc.tensor.matmul(pm,
                                             lhsT=w2_sb[:, (di * 3 + dj) * 2 + e, :],
                                             rhs=rhs, start=(ti == 0),
                                             stop=(ti == 17))
                            ti += 1
                half = 0 if j2 < 4 else 1
                m2 = m2g[j2 % 4]
                base = 1 + half * 64
                dst = bass.AP(tensor=m2.tensor, offset=m2.offset + base,
                              ap=[m2.ap[0], [128, 8], [1, 64]])
                nc.scalar.activation(out=dst,
                                     in_=pm[:, :].rearrange("p (r n) -> p r n", n=64),
                                     func=mybir.ActivationFunctionType.Exp,
                                     bias=b2_sb[:, :], scale=sc_sb[:, :])

            w3f_g, w3g_g = {}, {}

            def emit_weights(g):
                k0 = g * GB
                m2 = m2g[g]
                w3f = wrep.tile([128, GB, 3, 25], F32, name=f"w3f{g}", tag="w3f")
                for b2 in range(GB // 2):
                    k = k0 + 2 * b2
                    kl = 2 * b2
                    pw = psW.tile([128, 2, 128], BF16, name=f"pw{k}", tag="pw")
                    for bb in range(2):
                        nc.tensor.transpose(
                            pw[:, bb, 0:25],
                            m2[:, 1 + (kl + bb) * 128:1 + (kl + bb + 1) * 128],
                            id_sb[:, :])
                        nc.tensor.transpose(
                            pw[:, bb, 32:57],
                            m2[:, 2 + (kl + bb) * 128:2 + (kl + bb + 1) * 128],
                            id_sb[:, :])
                        nc.tensor.transpose(
                            pw[:, bb, 64:89],
                            m2[:, (kl + bb) * 128:(kl + bb + 1) * 128],
                            id_sb[:, :])
                    v6 = bass.AP(tensor=pw.tensor, offset=pw.offset,
                                 ap=[pw.ap[0], [128, 2], [32, 3], [1, 25]])
                    r6 = wsb.tile([128, 6], F32, name=f"r6{k}", tag="r6")
                    r6v = bass.AP(tensor=r6.tensor, offset=r6.offset,
                                  ap=[r6.ap[0], [3, 2], [1, 3]])
                    nc.vector.tensor_reduce(out=r6v, in_=v6,
                                            axis=mybir.AxisListType.X,
                                            op=mybir.AluOpType.add)
                    nc.vector.reciprocal(out=r6, in_=r6)
                    dstf = bass.AP(tensor=w3f.tensor,
                                   offset=w3f.offset + 2 * b2 * 75,
                                   ap=[w3f.ap[0], [75, 2], [25, 3], [1, 25]])
                    in1f = bass.AP(tensor=r6.tensor, offset=r6.offset,
                                   ap=[r6.ap[0], [3, 2], [1, 3], [0, 25]])
                    nc.vector.tensor_tensor(out=dstf, in0=v6, in1=in1f,
                                            op=mybir.AluOpType.mult)
                w3g = wrep.tile([128, GB, 3, 25, 2], BF16, name=f"w3g{g}",
                                tag="w3g")
                rep_dst = bass.AP(tensor=w3g.tensor, offset=w3g.offset,
                                  ap=[w3g.ap[0], [2, 600], [1, 2]])
                rep_src = bass.AP(tensor=w3f.tensor, offset=w3f.offset,
                                  ap=[w3f.ap[0], [1, 600], [0, 2]])
                nc.scalar.copy(out=rep_dst, in_=rep_src)
                w3f_g[g], w3g_g[g] = w3f, w3g

            def emit_group(g):
                k0 = g * GB
                w3f, w3g = w3f_g[g], w3g_g[g]

                def emit_product(tap, mode):
                    i, j = tap
                    cp, dj = _CPDJ[j]
                    oh, dh = _OHDH[i]
                    t = i * 5 + j
                    var = _VAR[dj]
                    s0 = _slot(k0 + dh, oh, cp)
                    pt = prodp.tile([128, GB, C], BF16, name=f"p{g}_{t}",
                                    tag="prod")
                    if mode == "a":
                        for b in range(GB):
                            scb = bass.AP(
                                tensor=w3f.tensor,
                                offset=w3f.offset + b * 75 + var * 25 + t,
                                ap=[w3f.ap[0], [1, 1]])
                            nc.scalar.activation(
                                out=pt[:, b, :],
                                in_=xall[:, s0 + 4 * b, :],
                                func=mybir.ActivationFunctionType.Copy,
                                scale=scb)
                        return pt
                    in0 = bass.AP(tensor=xall.tensor,
                                  offset=xall.offset + s0 * C,
                                  ap=[xall.ap[0], [4 * C, GB],
                                      [2, C // 2], [1, 2]])
                    in1 = bass.AP(tensor=w3g.tensor,
                                  offset=w3g.offset + var * 50 + t * 2,
                                  ap=[w3g.ap[0], [150, GB], [0, C // 2],
                                      [1, 2]])
                    outp = bass.AP(tensor=pt.tensor, offset=pt.offset,
                                   ap=[pt.ap[0], [C, GB], [2, C // 2], [1, 2]])
                    if mode == "g":
                        nc.gpsimd.tensor_tensor(out=outp, in0=in0, in1=in1,
                                                op=mybir.AluOpType.mult)
                    else:
                        nc.vector.tensor_tensor(out=outp, in0=in0, in1=in1,
                                                op=mybir.AluOpType.mult)
                    return pt

                po_list = [psP.tile([128, 512], F32, name=f"po{g}_{pr}",
                                    tag="po") for pr in range(GB // 2)]
                started = [False] * (GB // 2)
                paired = {t for pr_ in PAIRS for t in pr_}
                # slow producers (GPSIMD) build their products up front
                early = {}
                for tap in mm_taps:
                    if TAP_MODE[tap] == "g" and tap not in paired:
                        early[tap] = emit_product(tap, TAP_MODE[tap])
                # units: JIT v work first (identity, then shifted), ACT
                # products spread mid-stream, early GPSIMD consumed last
                units = ([(_VAR[_CPDJ[t1[1]][1]], (t1, t2)) for t1, t2 in PAIRS]
                         + [(_VAR[_CPDJ[t[1]][1]], (t,)) for t in mm_taps
                            if t not in paired])
                units.sort(key=lambda u: (u[1][0] in early,
                                          TAP_MODE[u[1][0]] == "a",
                                          u[0] != 0, u[0]))
                # move each a-unit to sit after every 3rd v-unit
                a_units = [u for u in units if TAP_MODE[u[1][0]] == "a"]
                o_units = [u for u in units if TAP_MODE[u[1][0]] != "a"]
                units = []
                ai = 0
                for n, u in enumerate(o_units):
                    units.append(u)
                    if n % 3 == 2 and ai < len(a_units):
                        units.append(a_units[ai])
                        ai += 1
                units.extend(a_units[ai:])
                for nu, (var, tt) in enumerate(units):
                    if len(tt) == 1:
                        pt = (early[tt[0]] if tt[0] in early
                              else emit_product(tt[0], TAP_MODE[tt[0]]))
                    else:
                        p1 = emit_product(tt[0], TAP_MODE[tt[0]])
                        p2 = emit_product(tt[1], TAP_MODE[tt[1]])
                        pt = prodp.tile([128, GB, C], BF16,
                                        name=f"ps{g}_{tt[0][0] * 5 + tt[0][1]}",
                                        tag="prod")
                        nc.vector.tensor_tensor(out=pt, in0=p1, in1=p2,
                                                op=mybir.AluOpType.add)
                    last = nu == len(units) - 1
                    for pr in range(GB // 2):
                        nc.tensor.matmul(po_list[pr], lhsT=shm_sb[:, var, :],
                                         rhs=pt[:, 2 * pr:2 * pr + 2, :],
                                         start=not started[pr], stop=last)
                        started[pr] = True

                for pr in range(GB // 2):
                    po = po_list[pr]
                    k = k0 + 2 * pr
                    st = outst0 if k < 16 else outst1
                    dst = st[:, (k % 16) * C:(k % 16 + 2) * C]
                    if pr % 2 == 0:
                        nc.vector.tensor_copy(out=dst, in_=po)
                    else:
                        nc.scalar.copy(out=dst, in_=po)

                if g == 1:
                    nc.sync.dma_start(out=out_d[:, 0:16 * C], in_=outst0)
                if g == 3:
                    nc.sync.dma_start(out=out_d[:, 16 * C:NB * C], in_=outst1)

            # ---- staggered schedule ----
            emit_chunk(0)
            emit_chunk(4)
            emit_weights(0)
            emit_chunk(1)
            emit_chunk(5)
            emit_weights(1)
            emit_group(0)
            emit_chunk(2)
            emit_chunk(6)
            emit_weights(2)
            emit_group(1)
            emit_chunk(3)
            emit_chunk(7)
            emit_weights(3)
            emit_group(2)
            emit_group(3)

    nc.compile()
    return nc


_NC_CACHE = None
LAST_RESULTS = None


def _get_nc():
    global _NC_CACHE
    if _NC_CACHE is None:
        _NC_CACHE = _build_nc()
    return _NC_CACHE


def _host_prep(x, w_comp, b_comp, w_enc, b_enc, power_p):
    pe = float(np.exp(np.float64(power_p)))

    xb = x.astype(NPBF)
    X_all = np.zeros((B, 128, 136, C), dtype=NPBF)
    for oh in range(2):
        for cp in range(2):
            g = xb[:, :, :, cp::2]
            for h in range(2):
                kks = [kk for kk in range(-1, 33)
                       if 0 <= 2 * (kk + 32 * h) + oh < H]
                rows = [2 * (kk + 32 * h) + oh for kk in kks]
                slots = [_slot(kk, oh, cp) for kk in kks]
                sub = g[:, :, rows, :].transpose(0, 3, 2, 1)
                X_all[:, 64 * h:64 * h + 64, slots, :] = sub

    XCPL = 16912 if DR else 16900
    xpad = np.zeros((B, 128, 2, XCPL), dtype=NPF8)
    xp = np.pad(x, ((0, 0), (0, 0), (1, 1), (1, 1))).astype(NPF8)
    for e in range(2):
        xpad[:, :, e, :16900] = \
            xp[:, e * 128:(e + 1) * 128].reshape(B, 128, 16900)
    xpad = xpad.reshape(B, 128, 2 * XCPL)

    wc = w_comp[:, :, 0, 0].astype(np.float64)
    W2 = np.einsum('tkij,kc->tijc', w_enc.astype(np.float64), wc)
    bias2 = b_enc.astype(np.float64) + \
        w_enc.astype(np.float64).sum(axis=(2, 3)) @ b_comp.astype(np.float64)
    amax = max(np.abs(W2).max(), 1e-30)
    SCALE = 2.0 ** np.floor(np.log2(192.0 / amax))
    if DR:
        w2s = np.zeros((128, 9, 2, 32), dtype=NPF8)
        for di in range(3):
            for dj in range(3):
                for e in range(2):
                    w2s[:, di * 3 + dj, e, :25] = \
                        (W2[:, di, dj, e * 128:(e + 1) * 128].T * SCALE).astype(NPF8)
    else:
        w2s = np.zeros((128, 18, 25), dtype=NPF8)
        for di in range(3):
            for dj in range(3):
                for e in range(2):
                    w2s[:, (di * 3 + dj) * 2 + e, :] = \
                        (W2[:, di, dj, e * 128:(e + 1) * 128].T * SCALE).astype(NPF8)
    b2 = (pe * bias2).reshape(25, 1).astype(np.float32)
    sc = np.full((25, 1), pe / SCALE, dtype=np.float32)
    idn = np.eye(25, dtype=NPBF)

    shm = np.zeros((128, 3, 128), dtype=NPBF)
    shm[np.arange(128), 0, np.arange(128)] = 1
    shm[np.arange(127), 1, np.arange(127) + 1] = 1
    shm[63, 1, 64] = 0
    shm[np.arange(1, 128), 2, np.arange(1, 128) - 1] = 1
    shm[64, 2, 63] = 0

    in_maps = []
    for b in range(B):
        in_maps.append({
            "xall": np.ascontiguousarray(X_all[b]),
            "xcp": np.ascontiguousarray(xpad[b]),
            "w2": w2s, "b2": b2, "sc": sc, "idn": idn, "shm": shm,
        })
    return in_maps


def kernel(x, w_comp, b_comp, w_enc, b_enc, power_p):
    x = np.asarray(x, dtype=np.float32)
    in_maps = _host_prep(x, np.asarray(w_comp), np.asarray(b_comp),
                         np.asarray(w_enc), np.asarray(b_enc),
                         np.asarray(power_p))
    nc = _get_nc()
    res = run_bass_kernel_spmd(nc, in_maps, list(range(NCORES)))
    global LAST_RESULTS
    LAST_RESULTS = res
    outs = np.stack([np.asarray(res.results[i]["out"]) for i in range(NCORES)])
    o = outs.reshape(B, 2, 64, NB, C).astype(np.float32)
    out = np.zeros((B, C, HP, WP), dtype=np.float32)
    for h in range(2):
        out[:, :, 32 * h:32 * h + 32, :] = o[:, h].transpose(0, 3, 2, 1)
    return np.ascontiguousarray(out)

